# revision 42
# baseline (speedup 1.0000x reference)
"""Local (sliding-window) attention kernel for TRN2, 8 NeuronCores.

Sharding: core c -> batch b=c//4, head-group hg=c%4 (4 heads of 16).
Each core computes qkv projection for its heads, banded attention, and a
partial out-projection (its heads' columns of Wo). Host sums the 4
partials per batch and adds bo.

All matmul operands bf16 (2x moving-operand stream rate vs fp32r, and
eligible for the full 2.4GHz HAM-warm PE clock); fp32 PSUM accumulate.

Device algorithm (per core):
  qkT[512,2048]  = wqk.T @ xT          (Q rows pre-scaled by 1/sqrt(hd))
  V  [2048,260]  = xT.T @ wv           (token-major; +bias, with a ones
                                        column per head -> vaug)
  per head h, key-block j (128 keys):
    S^T[k,q]     = kT_hj.T @ qT (q-window = 384 cols: blocks j..j+2)
    P^T          = exp(S^T) (bf16), zero band-complement triangles
                   (one strided DVE op)
    yT_psum[65,512] += vaug_hj.T @ P^T   (row 64 = softmax denominator)
  per (h, q-range g of 512), staggered across later ticks so no engine
  FIFO head-blocks:
    dn[1,512]  = denom row copied to partition 0 (ACT)
    rec        = reciprocal_approx_fast(dn) (custom DVE op)
    bc[64,512] = partition_broadcast(rec) (GpSimd)
    yT (bf16)  = yT_psum * bc (DVE)
  out[2048,1024] = yT.T @ wo  (partial; host adds across head-groups + bo)

Schedule: attention runs in g-major blocks (all heads for one 512-query
range) with crossing pV contributions deferred via retained P^T tiles
and replayed in the next block, so out-projection + output DMA fire at
every block tail, keeping PE duty (and the HAM clock gate) up. The
first block (EARLY) is interleaved into the qk/V projection of tokens
1024-2047 so ACT's exp work overlaps the PE-dense GEMM phase.
"""

import os
import sys

import numpy as np

if "/opt/trn_rl_repo" not in sys.path:
    sys.path.insert(0, "/opt/trn_rl_repo")

B, T, D = 2, 2048, 1024
H, W = 16, 256
HD = D // H          # 64
NCORES = 8
HPC = 4              # heads per core
FB = HPC * HD        # 256 f-columns per core

_STATE: dict = {}


def _build_module():
    import concourse.bacc as bacc
    import concourse.tile as tile
    from concourse import mybir

    dt = mybir.dt
    AF = mybir.ActivationFunctionType
    OP = mybir.AluOpType

    nc = bacc.Bacc(
        "TRN2",
        target_bir_lowering=False,
        debug=False,
        enable_asserts=False,
        num_devices=NCORES,
    )

    f32 = dt.float32
    f32r = dt.float32r
    bf16 = dt.bfloat16
    xT_d = nc.dram_tensor("xT", [D, T], bf16, kind="ExternalInput").ap()
    wqk_d = nc.dram_tensor("wqk", [D, 2 * FB], bf16, kind="ExternalInput").ap()
    bqk_d = nc.dram_tensor("bqk", [128, 4], f32, kind="ExternalInput").ap()
    wv_d = nc.dram_tensor("wv", [D, FB], bf16, kind="ExternalInput").ap()
    bvb_d = nc.dram_tensor("bvb", [128, HPC, HD], f32, kind="ExternalInput").ap()
    wo_d = nc.dram_tensor("wo", [FB, D], bf16, kind="ExternalInput").ap()
    tris_d = nc.dram_tensor("tris", [128, 256], bf16, kind="ExternalInput").ap()
    out_d = nc.dram_tensor("out_p", [T, D], bf16, kind="ExternalOutput").ap()

    KC = D // 128     # 8 contraction chunks
    NT = T // 128     # 16 token tiles / key blocks
    NQ = T // 512     # 4 q-ranges

    with tile.TileContext(nc) as tc:
        with (
            tc.tile_pool(name="const", bufs=1) as cpool,
            tc.tile_pool(name="work", bufs=3) as wpool,
            tc.tile_pool(name="ps", bufs=8, space="PSUM") as ppool,
        ):
            # ---- persistent SBUF ----
            # per-contraction-chunk tiles so each matmul depends only on
            # its own chunk's DMA, not the whole operand
            xTa_t = [cpool.tile([128, 1024], bf16, name=f"xTa{a}")
                     for a in range(KC)]
            xTb_t = [cpool.tile([128, 1024], bf16, name=f"xTb{a}")
                     for a in range(KC)]
            wqk_t = [cpool.tile([128, 2 * FB], bf16, name=f"wqk{a}")
                     for a in range(KC)]
            wv_t = [cpool.tile([128, FB], bf16, name=f"wv{a}")
                    for a in range(KC)]
            wo_sb = cpool.tile([128, 2, D], bf16)
            bqk_sb = cpool.tile([128, 4], f32)
            bvb_sb = cpool.tile([128, HPC, HD], f32)
            tris_sb = cpool.tile([128, 256], bf16)
            qkT_sb = cpool.tile([128, 4, T], bf16)
            vaug_sb = cpool.tile([128, NT, HPC, HD + 1], bf16)
            yTn_sb = cpool.tile([128, 2, T], bf16)

            # vaug ones columns via memset (DVE is idle during load)
            for h in range(HPC):
                nc.vector.memset(vaug_sb[:, :, h, HD:HD + 1], 1.0)

            # Two DMA queues (SP + ACT hwdge): group-0 operands (wqk, xTa)
            # lead both queues so the first matmul can start ~1us in;
            # constants follow, then second-half operands. ACT is idle
            # during the load phase.
            for a in range(KC):
                nc.sync.dma_start(wqk_t[a][:], wqk_d[a * 128:(a + 1) * 128, :])
                eng = nc.scalar if a % 2 == 0 else nc.gpsimd
                eng.dma_start(
                    xTa_t[a][:], xT_d[a * 128:(a + 1) * 128, 0:1024]
                )
            nc.sync.dma_start(bqk_sb[:], bqk_d[:])
            nc.sync.dma_start(bvb_sb[:], bvb_d[:])
            nc.sync.dma_start(tris_sb[:], tris_d[:])
            for a in range(KC):
                nc.sync.dma_start(wv_t[a][:], wv_d[a * 128:(a + 1) * 128, :])
            for a in range(KC):
                eng = nc.scalar if a % 2 == 0 else nc.gpsimd
                eng.dma_start(
                    xTb_t[a][:], xT_d[a * 128:(a + 1) * 128, 1024:2048]
                )
            for f in range(2):
                nc.sync.dma_start(wo_sb[:, f, :], wo_d[f * 128:(f + 1) * 128, :])

            # ---- qkT projection: [512, 2048] ----
            def qkT_pass(grp, xh_t, mm):
                # one m-pair pass of 4 PSUM tiles: enough bank distance to
                # avoid same-bank accumulation hazards, while leaving 4
                # ring slots for interleaved attention tiles. Pass mm=0
                # covers m={0,2} = q+k rows of heads 0-1, so their
                # attention can start after half the group.
                tiles = [(m, n) for m in (mm, mm + 2) for n in range(2)]
                ps_g = {
                    mn: ppool.tile([128, 512], f32, tag="ps",
                                   name=f"ps_qk{grp}_{mn[0]}_{mn[1]}")
                    for mn in tiles
                }
                for a in range(KC):
                    for (m, n) in tiles:
                        nc.tensor.matmul(
                            ps_g[(m, n)][:],
                            lhsT=wqk_t[a][:, m * 128:(m + 1) * 128],
                            rhs=xh_t[a][:, n * 512:(n + 1) * 512],
                            start=(a == 0),
                            stop=(a == KC - 1),
                        )
                for (m, n) in tiles:
                    nc.scalar.activation(
                        qkT_sb[:, m,
                               (2 * grp + n) * 512:(2 * grp + n + 1) * 512],
                        ps_g[(m, n)][:],
                        AF.Identity,
                        bias=bqk_sb[:, m:m + 1],
                    )

            def v_proj(t, xh_t):
                tl = t % 8
                ps_v = ppool.tile([128, HPC, HD], f32, tag="ps", name=f"ps_v_{t}")
                for a in range(KC):
                    nc.tensor.matmul(
                        ps_v[:],
                        lhsT=xh_t[a][:, tl * 128:(tl + 1) * 128],
                        rhs=wv_t[a][:],
                        start=(a == 0),
                        stop=(a == KC - 1),
                    )
                nc.vector.tensor_tensor(
                    out=vaug_sb[:, t, :, 0:HD],
                    in0=ps_v[:],
                    in1=bvb_sb[:],
                    op=OP.add,
                )

            qkT_pass(0, xTa_t, 0)
            for t in range(8):
                v_proj(t, xTa_t)

            # ---- attention: software-pipelined over flattened (h, j) ----
            # stage A:  paired S^T matmuls (j even + odd) into a 2-bank
            #           PSUM tile -> ONE exp over both halves (strided AP)
            #           -> band-complement triangle masks per half (DVE)
            # stage B:  pV matmuls
            # Step order interleaves an EARLY batch (all h, j<=3, pV
            # clipped to q-range g=0) before group-1 projection, so ACT's
            # exp work overlaps the PE-dense projection of tokens
            # 1024-2047. The clipped g=1 contributions of j=2,3 replay
            # from retained pT tiles at each head's first REST step.
            # The (h,g)-tail normalize chain (denom copy -> approx recip ->
            # partition broadcast -> multiply) is staggered across later
            # ticks via `post`, so each op's inputs are already complete
            # when it reaches its engine's strict-FIFO head — otherwise
            # the chain head-blocks the DVE/GpSimd queues that the
            # per-step mask ops need, stalling the PE ~13us per group.
            DELAY = 4
            EARLY_N = 4 * HPC
            # g-major blocks: all heads' steps for one 512-query range,
            # crossing pV contributions deferred to the next block (replay
            # from retained pT). out-projection + output DMA then fire at
            # every block tail, keeping PE duty (and the HAM clock) up
            # through the whole attention phase.
            steps = [(h, j, g) for g in range(NQ) for h in range(HPC)
                     for j in range(4 * g, min(4 * g + 4, NT))]
            pT_t = {}
            ps_y = {}
            post = {}

            def at_step(s, fn):
                post.setdefault(s, []).append(fn)

            o_tiles = {}

            def out_proj_chunk(mt, nn):
                # one (mt, nn) output tile: 2 accum matmuls + copy (+ DMA
                # when the mt row completes). Chunks are spread across
                # ticks so the PE burst doesn't starve the exp pipeline.
                if nn == 0:
                    o_tiles[mt] = wpool.tile(
                        [128, 2, 512], bf16, bufs=6,
                        name=f"o_{mt}", tag="o_sb",
                    )
                o_sb = o_tiles[mt]
                ps_o = ppool.tile(
                    [128, 512], f32, tag="ps", name=f"ps_o_{mt}_{nn}",
                )
                for fc in range(2):
                    nc.tensor.matmul(
                        ps_o[:],
                        lhsT=yTn_sb[:, fc, mt * 128:(mt + 1) * 128],
                        rhs=wo_sb[:, fc, nn * 512:(nn + 1) * 512],
                        start=(fc == 0),
                        stop=(fc == 1),
                    )
                if (mt + nn) % 2 == 0:
                    nc.vector.tensor_copy(out=o_sb[:, nn, :], in_=ps_o[:])
                else:
                    nc.scalar.copy(o_sb[:, nn, :], ps_o[:])
                if nn == 1:
                    del o_tiles[mt]
                    eng = nc.sync if mt % 2 == 0 else nc.gpsimd
                    eng.dma_start(
                        out_d[mt * 128:(mt + 1) * 128, :], o_sb[:]
                    )

            def stage_a(k):
                h, j, _g = steps[k]
                po = 64 * (h % 2)
                qwin = min(384, T - 128 * j)
                ps_s = ppool.tile([128, 384], f32, tag="ps",
                                  name=f"ps_s_{k}")
                nc.tensor.matmul(
                    ps_s[:, :qwin],
                    lhsT=qkT_sb[po:po + 64, 2 + h // 2,
                                j * 128:(j + 1) * 128],
                    rhs=qkT_sb[po:po + 64, h // 2,
                               j * 128:j * 128 + qwin],
                    start=True,
                    stop=True,
                )
                pT = wpool.tile([128, 384], bf16, bufs=16,
                                name=f"pT_{k}", tag="pT")
                nc.scalar.activation(pT[:, :qwin], ps_s[:, :qwin], AF.Exp)
                if qwin == 384:
                    pv = pT[:].rearrange("p (a b) -> p a b", a=3)[:, 0:3:2, :]
                    tv = tris_sb[:].rearrange("p (a b) -> p a b", a=2)
                    nc.vector.tensor_tensor(
                        out=pv, in0=pv, in1=tv, op=OP.mult,
                    )
                else:
                    nc.vector.tensor_tensor(
                        out=pT[:, 0:128], in0=pT[:, 0:128],
                        in1=tris_sb[:, 0:128], op=OP.mult,
                    )
                pT_t[(h, j)] = pT

            def stage_b(idx, pidx):
                h, j, g = steps[idx]
                po = 64 * (h % 2)
                qwin = min(384, T - 128 * j)
                if g >= 1 and j == 4 * g:
                    # replay the parts of the previous block's crossing
                    # key-blocks that were clipped to their own g
                    ps_y[(h, g)] = ppool.tile(
                        [65, 512], f32, tag="ps", name=f"ps_y_{h}_{g}"
                    )
                    for jr in (4 * g - 2, 4 * g - 1):
                        pTr = pT_t.pop((h, jr))
                        c0 = 512 * g - 128 * jr
                        c1 = min(384, 512 * (g + 1) - 128 * jr)
                        nc.tensor.matmul(
                            ps_y[(h, g)][:, 0:c1 - c0],
                            lhsT=vaug_sb[:, jr, h, :],
                            rhs=pTr[:, c0:c1],
                            start=(jr == 4 * g - 2),
                            stop=False,
                            skip_group_check=True,
                        )
                pT = pT_t[(h, j)]
                if not (j >= 4 * g + 2 and g < NQ - 1):
                    del pT_t[(h, j)]
                if True:
                    c0 = 0
                    c1 = min(qwin, 512 * (g + 1) - 128 * j)
                    if (h, g) not in ps_y:
                        ps_y[(h, g)] = ppool.tile(
                            [65, 512], f32, tag="ps", name=f"ps_y_{h}_{g}"
                        )
                    first = (g == 0 and j == 0)
                    last = (j == min(NT - 1, 4 * g + 3))
                    d0 = 128 * j - 512 * g
                    nc.tensor.matmul(
                        ps_y[(h, g)][:, d0:d0 + (c1 - c0)],
                        lhsT=vaug_sb[:, j, h, :],
                        rhs=pT[:, c0:c1],
                        start=first,
                        stop=last,
                        skip_group_check=True,
                    )
                    if not last:
                        return
                    yps = ps_y.pop((h, g))
                    # reciprocal_approx_fast and partition_broadcast both
                    # require partition base 0: stage the denominator row
                    # at partition 0 first.
                    dn = wpool.tile([1, 512], f32, bufs=4,
                                    name=f"dn_{h}_{g}", tag="dn")
                    rec = wpool.tile([1, 512], f32, bufs=4,
                                     name=f"rec_{h}_{g}", tag="rec")
                    bc_sb = wpool.tile([64, 512], f32, bufs=3,
                                       name=f"bc_{h}_{g}", tag="bc")

                    def dn_copy(dn=dn, yps=yps):
                        nc.scalar.copy(dn[:], yps[64:65, :])

                    def do_recip(rec=rec, dn=dn):
                        nc.vector.reciprocal_approx_fast(rec[:], dn[:])

                    def do_bcast(bc_sb=bc_sb, rec=rec):
                        nc.gpsimd.partition_broadcast(bc_sb[:], rec[0:1, :])

                    def do_mult(yps=yps, bc_sb=bc_sb, po=po, h=h, g=g):
                        nc.vector.tensor_tensor(
                            out=yTn_sb[po:po + 64, h // 2,
                                       g * 512:(g + 1) * 512],
                            in0=yps[0:64, :],
                            in1=bc_sb[:],
                            op=OP.mult,
                        )

                    at_step(pidx + 1, dn_copy)
                    at_step(pidx + 2, do_recip)
                    at_step(pidx + 3, do_bcast)
                    at_step(pidx + 4, do_mult)
                    if h == HPC - 1:
                        for i, (mt, nn) in enumerate(
                                (mt, nn)
                                for mt in range(4 * g, 4 * g + 4)
                                for nn in range(2)):
                            at_step(
                                pidx + 4 + i // 2,
                                lambda mt=mt, nn=nn: out_proj_chunk(mt, nn),
                            )

            units = ["s"] * 8
            units += [lambda: qkT_pass(0, xTa_t, 1)]
            units += ["s"] * 8
            units += [(lambda t=t: v_proj(t, xTb_t)) for t in range(8, 12)]
            units += [lambda: qkT_pass(1, xTb_t, 0)]
            units += [(lambda t=t: v_proj(t, xTb_t)) for t in range(12, NT)]
            units += [lambda: qkT_pass(1, xTb_t, 1)]
            units += ["s"] * (len(steps) - EARLY_N)
            total_ticks = len(units) + DELAY + 10
            a_i = 0
            b_i = 0
            for tick in range(total_ticks):
                u = units[tick] if tick < len(units) else None
                is_emit = u is not None and u != "s"
                if u == "s":
                    stage_a(a_i)
                    a_i += 1
                elif is_emit:
                    u()
                if b_i < a_i and (a_i - b_i > DELAY or is_emit
                                  or u is None or a_i == len(steps)):
                    stage_b(b_i, tick)
                    b_i += 1
                for fn in post.pop(tick, []):
                    fn()

    nc.compile()
    from concourse.bass_interp import get_hw_module

    nc.m = get_hw_module(nc.m)
    return nc


def _shard_inputs(x, Wqkv, bqkv, Wo, bo):
    import ml_dtypes

    bfdt = ml_dtypes.bfloat16

    x = np.asarray(x, np.float32)
    Wqkv = np.asarray(Wqkv, np.float32)
    bqkv = np.asarray(bqkv, np.float32)
    Wo = np.asarray(Wo, np.float32)

    scale = 1.0 / np.sqrt(np.float32(HD))
    c_idx = np.arange(128)[:, None]
    u_idx = np.arange(128)[None, :]
    tri0 = (u_idx >= c_idx).astype(np.float32)   # keys block j vs q block j
    tri1 = (u_idx < c_idx).astype(np.float32)    # keys block j vs q block j+2
    tris = np.concatenate([tri0, tri1], axis=1)

    in_maps = []
    for c in range(NCORES):
        b, hg = divmod(c, HPC)
        r0 = hg * FB
        Wq = Wqkv[r0:r0 + FB] * scale
        Wk = Wqkv[D + r0:D + r0 + FB]
        Wv = Wqkv[2 * D + r0:2 * D + r0 + FB]
        bq = bqkv[r0:r0 + FB] * scale
        bk = bqkv[D + r0:D + r0 + FB]
        bv = bqkv[2 * D + r0:2 * D + r0 + FB]
        in_maps.append({
            "xT": np.ascontiguousarray(x[b].T).astype(bfdt),
            "wqk": np.ascontiguousarray(
                np.concatenate([Wq, Wk], 0).T).astype(bfdt),
            "bqk": np.ascontiguousarray(
                np.concatenate([bq, bk]).reshape(4, 128).T),
            "wv": np.ascontiguousarray(Wv.T).astype(bfdt),
            "bvb": np.ascontiguousarray(
                np.broadcast_to(bv[None, :], (128, FB))
            ).reshape(128, HPC, HD),
            "wo": np.ascontiguousarray(Wo[:, r0:r0 + FB].T).astype(bfdt),
            "tris": tris.astype(bfdt),
            "vone": np.ones((128, 64), bfdt),
        })
    return in_maps


def kernel(x, Wqkv, bqkv, Wo, bo):
    from concourse import bass_utils

    if "nc" not in _STATE:
        _STATE["nc"] = _build_module()
    nc = _STATE["nc"]

    in_maps = _shard_inputs(x, Wqkv, bqkv, Wo, bo)
    trace = bool(os.environ.get("TRNKERN_TRACE"))
    res = bass_utils.run_bass_kernel_spmd(
        nc,
        in_maps,
        core_ids=list(range(NCORES)),
        trace=trace,
    )
    _STATE["last"] = res

    bo = np.asarray(bo, np.float32)
    out = np.empty((B, T, D), np.float32)
    for b in range(B):
        acc = res.results[b * HPC]["out_p"].astype(np.float32)
        for hg in range(1, HPC):
            acc = acc + res.results[b * HPC + hg]["out_p"].astype(np.float32)
        out[b] = acc + bo[None, :]
    return out


# revision 43
# speedup vs baseline: 1.0056x; 1.0056x over previous
"""Local (sliding-window) attention kernel for TRN2, 8 NeuronCores.

Sharding: core c -> batch b=c//4, head-group hg=c%4 (4 heads of 16).
Each core computes qkv projection for its heads, banded attention, and a
partial out-projection (its heads' columns of Wo). Host sums the 4
partials per batch and adds bo.

All matmul operands bf16 (2x moving-operand stream rate vs fp32r, and
eligible for the full 2.4GHz HAM-warm PE clock); fp32 PSUM accumulate.

Device algorithm (per core):
  qkT[512,2048]  = wqk.T @ xT          (Q rows pre-scaled by 1/sqrt(hd))
  V  [2048,260]  = xT.T @ wv           (token-major; +bias, with a ones
                                        column per head -> vaug)
  per head h, key-block j (128 keys):
    S^T[k,q]     = kT_hj.T @ qT (q-window = 384 cols: blocks j..j+2)
    P^T          = exp(S^T) (bf16), zero band-complement triangles
                   (one strided DVE op)
    yT_psum[65,512] += vaug_hj.T @ P^T   (row 64 = softmax denominator)
  per (h, q-range g of 512), staggered across later ticks so no engine
  FIFO head-blocks:
    dn[1,512]  = denom row copied to partition 0 (ACT)
    rec        = reciprocal_approx_fast(dn) (custom DVE op)
    bc[64,512] = partition_broadcast(rec) (GpSimd)
    yT (bf16)  = yT_psum * bc (DVE)
  out[2048,1024] = yT.T @ wo  (partial; host adds across head-groups + bo)

Schedule: attention runs in g-major blocks (all heads for one 512-query
range) with crossing pV contributions deferred via retained P^T tiles
and replayed in the next block, so out-projection + output DMA fire at
every block tail, keeping PE duty (and the HAM clock gate) up. The
first block (EARLY) is interleaved into the qk/V projection of tokens
1024-2047 so ACT's exp work overlaps the PE-dense GEMM phase.
"""

import os
import sys

import numpy as np

if "/opt/trn_rl_repo" not in sys.path:
    sys.path.insert(0, "/opt/trn_rl_repo")

B, T, D = 2, 2048, 1024
H, W = 16, 256
HD = D // H          # 64
NCORES = 8
HPC = 4              # heads per core
FB = HPC * HD        # 256 f-columns per core

_STATE: dict = {}


def _build_module():
    import concourse.bacc as bacc
    import concourse.tile as tile
    from concourse import mybir

    dt = mybir.dt
    AF = mybir.ActivationFunctionType
    OP = mybir.AluOpType

    nc = bacc.Bacc(
        "TRN2",
        target_bir_lowering=False,
        debug=False,
        enable_asserts=False,
        num_devices=NCORES,
    )

    f32 = dt.float32
    f32r = dt.float32r
    bf16 = dt.bfloat16
    xT_d = nc.dram_tensor("xT", [D, T], bf16, kind="ExternalInput").ap()
    wqk_d = nc.dram_tensor("wqk", [D, 2 * FB], bf16, kind="ExternalInput").ap()
    bqk_d = nc.dram_tensor("bqk", [128, 4], f32, kind="ExternalInput").ap()
    wv_d = nc.dram_tensor("wv", [D, FB], bf16, kind="ExternalInput").ap()
    bvb_d = nc.dram_tensor("bvb", [128, HPC, HD], f32, kind="ExternalInput").ap()
    wo_d = nc.dram_tensor("wo", [FB, D], bf16, kind="ExternalInput").ap()
    tris_d = nc.dram_tensor("tris", [128, 256], bf16, kind="ExternalInput").ap()
    out_d = nc.dram_tensor("out_p", [T, D], bf16, kind="ExternalOutput").ap()

    KC = D // 128     # 8 contraction chunks
    NT = T // 128     # 16 token tiles / key blocks
    NQ = T // 512     # 4 q-ranges

    with tile.TileContext(nc) as tc:
        with (
            tc.tile_pool(name="const", bufs=1) as cpool,
            tc.tile_pool(name="work", bufs=3) as wpool,
            tc.tile_pool(name="ps", bufs=8, space="PSUM") as ppool,
        ):
            # ---- persistent SBUF ----
            # per-contraction-chunk tiles so each matmul depends only on
            # its own chunk's DMA, not the whole operand
            xTa_t = [cpool.tile([128, 1024], bf16, name=f"xTa{a}")
                     for a in range(KC)]
            xTb_t = [cpool.tile([128, 1024], bf16, name=f"xTb{a}")
                     for a in range(KC)]
            wqk_t = [cpool.tile([128, 2 * FB], bf16, name=f"wqk{a}")
                     for a in range(KC)]
            wv_t = [cpool.tile([128, FB], bf16, name=f"wv{a}")
                    for a in range(KC)]
            wo_sb = cpool.tile([128, 2, D], bf16)
            bqk_sb = cpool.tile([128, 4], f32)
            bvb_sb = cpool.tile([128, HPC, HD], f32)
            tris_sb = cpool.tile([128, 256], bf16)
            qkT_sb = cpool.tile([128, 4, T], bf16)
            vaug_sb = cpool.tile([128, NT, HPC, HD + 1], bf16)
            yTn_sb = cpool.tile([128, 2, T], bf16)

            # vaug ones columns via memset (DVE is idle during load)
            for h in range(HPC):
                nc.vector.memset(vaug_sb[:, :, h, HD:HD + 1], 1.0)

            # Two DMA queues (SP + ACT hwdge): group-0 operands (wqk, xTa)
            # lead both queues so the first matmul can start ~1us in;
            # constants follow, then second-half operands. ACT is idle
            # during the load phase.
            for a in range(KC):
                nc.sync.dma_start(wqk_t[a][:], wqk_d[a * 128:(a + 1) * 128, :])
                eng = nc.scalar if a % 2 == 0 else nc.gpsimd
                eng.dma_start(
                    xTa_t[a][:], xT_d[a * 128:(a + 1) * 128, 0:1024]
                )
            nc.sync.dma_start(bqk_sb[:], bqk_d[:])
            nc.sync.dma_start(bvb_sb[:], bvb_d[:])
            nc.sync.dma_start(tris_sb[:], tris_d[:])
            for a in range(KC):
                nc.sync.dma_start(wv_t[a][:], wv_d[a * 128:(a + 1) * 128, :])
            for a in range(KC):
                eng = nc.scalar if a % 2 == 0 else nc.gpsimd
                eng.dma_start(
                    xTb_t[a][:], xT_d[a * 128:(a + 1) * 128, 1024:2048]
                )
            for f in range(2):
                nc.sync.dma_start(wo_sb[:, f, :], wo_d[f * 128:(f + 1) * 128, :])

            # ---- qkT projection: [512, 2048] ----
            def qkT_pass(grp, xh_t, mm):
                # one m-pair pass of 4 PSUM tiles: enough bank distance to
                # avoid same-bank accumulation hazards, while leaving 4
                # ring slots for interleaved attention tiles. Pass mm=0
                # covers m={0,2} = q+k rows of heads 0-1, so their
                # attention can start after half the group.
                tiles = [(m, n) for m in (mm, mm + 2) for n in range(2)]
                ps_g = {
                    mn: ppool.tile([128, 512], f32, tag="ps",
                                   name=f"ps_qk{grp}_{mn[0]}_{mn[1]}")
                    for mn in tiles
                }
                for a in range(KC):
                    for (m, n) in tiles:
                        nc.tensor.matmul(
                            ps_g[(m, n)][:],
                            lhsT=wqk_t[a][:, m * 128:(m + 1) * 128],
                            rhs=xh_t[a][:, n * 512:(n + 1) * 512],
                            start=(a == 0),
                            stop=(a == KC - 1),
                        )
                for (m, n) in tiles:
                    nc.scalar.activation(
                        qkT_sb[:, m,
                               (2 * grp + n) * 512:(2 * grp + n + 1) * 512],
                        ps_g[(m, n)][:],
                        AF.Identity,
                        bias=bqk_sb[:, m:m + 1],
                    )

            def v_proj(t, xh_t):
                tl = t % 8
                ps_v = ppool.tile([128, HPC, HD], f32, tag="ps", name=f"ps_v_{t}")
                for a in range(KC):
                    nc.tensor.matmul(
                        ps_v[:],
                        lhsT=xh_t[a][:, tl * 128:(tl + 1) * 128],
                        rhs=wv_t[a][:],
                        start=(a == 0),
                        stop=(a == KC - 1),
                    )
                nc.vector.tensor_tensor(
                    out=vaug_sb[:, t, :, 0:HD],
                    in0=ps_v[:],
                    in1=bvb_sb[:],
                    op=OP.add,
                )

            qkT_pass(0, xTa_t, 0)
            for t in range(8):
                v_proj(t, xTa_t)

            # ---- attention: software-pipelined over flattened (h, j) ----
            # stage A:  paired S^T matmuls (j even + odd) into a 2-bank
            #           PSUM tile -> ONE exp over both halves (strided AP)
            #           -> band-complement triangle masks per half (DVE)
            # stage B:  pV matmuls
            # Step order interleaves an EARLY batch (all h, j<=3, pV
            # clipped to q-range g=0) before group-1 projection, so ACT's
            # exp work overlaps the PE-dense projection of tokens
            # 1024-2047. The clipped g=1 contributions of j=2,3 replay
            # from retained pT tiles at each head's first REST step.
            # The (h,g)-tail normalize chain (denom copy -> approx recip ->
            # partition broadcast -> multiply) is staggered across later
            # ticks via `post`, so each op's inputs are already complete
            # when it reaches its engine's strict-FIFO head — otherwise
            # the chain head-blocks the DVE/GpSimd queues that the
            # per-step mask ops need, stalling the PE ~13us per group.
            DELAY = 4
            EARLY_N = 4 * HPC
            # g-major blocks: all heads' steps for one 512-query range,
            # crossing pV contributions deferred to the next block (replay
            # from retained pT). out-projection + output DMA then fire at
            # every block tail, keeping PE duty (and the HAM clock) up
            # through the whole attention phase.
            steps = [(h, j, g) for g in range(NQ) for h in range(HPC)
                     for j in range(4 * g, min(4 * g + 4, NT))]
            pT_t = {}
            ps_y = {}
            post = {}

            def at_step(s, fn):
                post.setdefault(s, []).append(fn)

            o_tiles = {}

            def out_proj_chunk(mt, nn):
                # one (mt, nn) output tile: 2 accum matmuls + copy (+ DMA
                # when the mt row completes). Chunks are spread across
                # ticks so the PE burst doesn't starve the exp pipeline.
                if nn == 0:
                    o_tiles[mt] = wpool.tile(
                        [128, 2, 512], bf16, bufs=6,
                        name=f"o_{mt}", tag="o_sb",
                    )
                o_sb = o_tiles[mt]
                ps_o = ppool.tile(
                    [128, 512], f32, tag="ps", name=f"ps_o_{mt}_{nn}",
                )
                for fc in range(2):
                    nc.tensor.matmul(
                        ps_o[:],
                        lhsT=yTn_sb[:, fc, mt * 128:(mt + 1) * 128],
                        rhs=wo_sb[:, fc, nn * 512:(nn + 1) * 512],
                        start=(fc == 0),
                        stop=(fc == 1),
                    )
                if (mt + nn) % 2 == 0:
                    nc.vector.tensor_copy(out=o_sb[:, nn, :], in_=ps_o[:])
                else:
                    nc.scalar.copy(o_sb[:, nn, :], ps_o[:])
                if nn == 1:
                    del o_tiles[mt]
                    eng = nc.sync if mt % 2 == 0 else nc.gpsimd
                    eng.dma_start(
                        out_d[mt * 128:(mt + 1) * 128, :], o_sb[:]
                    )

            def stage_a(k):
                h, j, _g = steps[k]
                po = 64 * (h % 2)
                qwin = min(384, T - 128 * j)
                ps_s = ppool.tile([128, 384], f32, tag="ps",
                                  name=f"ps_s_{k}")
                nc.tensor.matmul(
                    ps_s[:, :qwin],
                    lhsT=qkT_sb[po:po + 64, 2 + h // 2,
                                j * 128:(j + 1) * 128],
                    rhs=qkT_sb[po:po + 64, h // 2,
                               j * 128:j * 128 + qwin],
                    start=True,
                    stop=True,
                )
                pT = wpool.tile([128, 384], bf16, bufs=16,
                                name=f"pT_{k}", tag="pT")
                nc.scalar.activation(pT[:, :qwin], ps_s[:, :qwin], AF.Exp)
                if qwin == 384:
                    pv = pT[:].rearrange("p (a b) -> p a b", a=3)[:, 0:3:2, :]
                    tv = tris_sb[:].rearrange("p (a b) -> p a b", a=2)
                    nc.vector.tensor_tensor(
                        out=pv, in0=pv, in1=tv, op=OP.mult,
                    )
                else:
                    nc.vector.tensor_tensor(
                        out=pT[:, 0:128], in0=pT[:, 0:128],
                        in1=tris_sb[:, 0:128], op=OP.mult,
                    )
                pT_t[(h, j)] = pT

            def stage_b(idx, pidx):
                h, j, g = steps[idx]
                po = 64 * (h % 2)
                qwin = min(384, T - 128 * j)
                if g >= 1 and j == 4 * g:
                    # replay the parts of the previous block's crossing
                    # key-blocks that were clipped to their own g
                    ps_y[(h, g)] = ppool.tile(
                        [65, 512], f32, tag="ps", name=f"ps_y_{h}_{g}"
                    )
                    for jr in (4 * g - 2, 4 * g - 1):
                        pTr = pT_t.pop((h, jr))
                        c0 = 512 * g - 128 * jr
                        c1 = min(384, 512 * (g + 1) - 128 * jr)
                        nc.tensor.matmul(
                            ps_y[(h, g)][:, 0:c1 - c0],
                            lhsT=vaug_sb[:, jr, h, :],
                            rhs=pTr[:, c0:c1],
                            start=(jr == 4 * g - 2),
                            stop=False,
                            skip_group_check=True,
                        )
                pT = pT_t[(h, j)]
                if not (j >= 4 * g + 2 and g < NQ - 1):
                    del pT_t[(h, j)]
                if True:
                    c0 = 0
                    c1 = min(qwin, 512 * (g + 1) - 128 * j)
                    if (h, g) not in ps_y:
                        ps_y[(h, g)] = ppool.tile(
                            [65, 512], f32, tag="ps", name=f"ps_y_{h}_{g}"
                        )
                    first = (g == 0 and j == 0)
                    last = (j == min(NT - 1, 4 * g + 3))
                    d0 = 128 * j - 512 * g
                    nc.tensor.matmul(
                        ps_y[(h, g)][:, d0:d0 + (c1 - c0)],
                        lhsT=vaug_sb[:, j, h, :],
                        rhs=pT[:, c0:c1],
                        start=first,
                        stop=last,
                        skip_group_check=True,
                    )
                    if not last:
                        return
                    yps = ps_y.pop((h, g))
                    # reciprocal_approx_fast and partition_broadcast both
                    # require partition base 0: stage the denominator row
                    # at partition 0 first.
                    dn = wpool.tile([1, 512], f32, bufs=4,
                                    name=f"dn_{h}_{g}", tag="dn")
                    rec = wpool.tile([1, 512], f32, bufs=4,
                                     name=f"rec_{h}_{g}", tag="rec")
                    bc_sb = wpool.tile([64, 512], f32, bufs=3,
                                       name=f"bc_{h}_{g}", tag="bc")

                    def dn_copy(dn=dn, yps=yps):
                        nc.scalar.copy(dn[:], yps[64:65, :])

                    def do_recip(rec=rec, dn=dn):
                        nc.vector.reciprocal_approx_fast(rec[:], dn[:])

                    def do_bcast(bc_sb=bc_sb, rec=rec):
                        nc.gpsimd.partition_broadcast(bc_sb[:], rec[0:1, :])

                    def do_mult(yps=yps, bc_sb=bc_sb, po=po, h=h, g=g):
                        nc.vector.tensor_tensor(
                            out=yTn_sb[po:po + 64, h // 2,
                                       g * 512:(g + 1) * 512],
                            in0=yps[0:64, :],
                            in1=bc_sb[:],
                            op=OP.mult,
                        )

                    at_step(pidx + 1, dn_copy)
                    at_step(pidx + 2, do_recip)
                    at_step(pidx + 3, do_bcast)
                    at_step(pidx + 4, do_mult)
                    if h == HPC - 1:
                        for i, (mt, nn) in enumerate(
                                (mt, nn)
                                for mt in range(4 * g, 4 * g + 4)
                                for nn in range(2)):
                            at_step(
                                pidx + 4 + i // 2,
                                lambda mt=mt, nn=nn: out_proj_chunk(mt, nn),
                            )

            units = ["s"] * 8
            units += [lambda: qkT_pass(0, xTa_t, 1)]
            units += ["s"] * 8
            units += [lambda: qkT_pass(1, xTb_t, 0),
                      lambda: qkT_pass(1, xTb_t, 1)]
            units += [(lambda t=t: v_proj(t, xTb_t)) for t in range(8, NT)]
            units += ["s"] * (len(steps) - EARLY_N)
            total_ticks = len(units) + DELAY + 10
            a_i = 0
            b_i = 0
            for tick in range(total_ticks):
                u = units[tick] if tick < len(units) else None
                is_emit = u is not None and u != "s"
                if u == "s":
                    stage_a(a_i)
                    a_i += 1
                elif is_emit:
                    u()
                if b_i < a_i and (a_i - b_i > DELAY or is_emit
                                  or u is None or a_i == len(steps)):
                    stage_b(b_i, tick)
                    b_i += 1
                for fn in post.pop(tick, []):
                    fn()

    nc.compile()
    from concourse.bass_interp import get_hw_module

    nc.m = get_hw_module(nc.m)
    return nc


def _shard_inputs(x, Wqkv, bqkv, Wo, bo):
    import ml_dtypes

    bfdt = ml_dtypes.bfloat16

    x = np.asarray(x, np.float32)
    Wqkv = np.asarray(Wqkv, np.float32)
    bqkv = np.asarray(bqkv, np.float32)
    Wo = np.asarray(Wo, np.float32)

    scale = 1.0 / np.sqrt(np.float32(HD))
    c_idx = np.arange(128)[:, None]
    u_idx = np.arange(128)[None, :]
    tri0 = (u_idx >= c_idx).astype(np.float32)   # keys block j vs q block j
    tri1 = (u_idx < c_idx).astype(np.float32)    # keys block j vs q block j+2
    tris = np.concatenate([tri0, tri1], axis=1)

    in_maps = []
    for c in range(NCORES):
        b, hg = divmod(c, HPC)
        r0 = hg * FB
        Wq = Wqkv[r0:r0 + FB] * scale
        Wk = Wqkv[D + r0:D + r0 + FB]
        Wv = Wqkv[2 * D + r0:2 * D + r0 + FB]
        bq = bqkv[r0:r0 + FB] * scale
        bk = bqkv[D + r0:D + r0 + FB]
        bv = bqkv[2 * D + r0:2 * D + r0 + FB]
        in_maps.append({
            "xT": np.ascontiguousarray(x[b].T).astype(bfdt),
            "wqk": np.ascontiguousarray(
                np.concatenate([Wq, Wk], 0).T).astype(bfdt),
            "bqk": np.ascontiguousarray(
                np.concatenate([bq, bk]).reshape(4, 128).T),
            "wv": np.ascontiguousarray(Wv.T).astype(bfdt),
            "bvb": np.ascontiguousarray(
                np.broadcast_to(bv[None, :], (128, FB))
            ).reshape(128, HPC, HD),
            "wo": np.ascontiguousarray(Wo[:, r0:r0 + FB].T).astype(bfdt),
            "tris": tris.astype(bfdt),
            "vone": np.ones((128, 64), bfdt),
        })
    return in_maps


def kernel(x, Wqkv, bqkv, Wo, bo):
    from concourse import bass_utils

    if "nc" not in _STATE:
        _STATE["nc"] = _build_module()
    nc = _STATE["nc"]

    in_maps = _shard_inputs(x, Wqkv, bqkv, Wo, bo)
    trace = bool(os.environ.get("TRNKERN_TRACE"))
    res = bass_utils.run_bass_kernel_spmd(
        nc,
        in_maps,
        core_ids=list(range(NCORES)),
        trace=trace,
    )
    _STATE["last"] = res

    bo = np.asarray(bo, np.float32)
    out = np.empty((B, T, D), np.float32)
    for b in range(B):
        acc = res.results[b * HPC]["out_p"].astype(np.float32)
        for hg in range(1, HPC):
            acc = acc + res.results[b * HPC + hg]["out_p"].astype(np.float32)
        out[b] = acc + bo[None, :]
    return out


# revision 44
# speedup vs baseline: 1.0199x; 1.0143x over previous
"""Local (sliding-window) attention kernel for TRN2, 8 NeuronCores.

Sharding: core c -> batch b=c//4, head-group hg=c%4 (4 heads of 16).
Each core computes qkv projection for its heads, banded attention, and a
partial out-projection (its heads' columns of Wo). Host sums the 4
partials per batch and adds bo.

All matmul operands bf16 (2x moving-operand stream rate vs fp32r, and
eligible for the full 2.4GHz HAM-warm PE clock); fp32 PSUM accumulate.

Device algorithm (per core):
  qkT[512,2048]  = wqk.T @ xT          (Q rows pre-scaled by 1/sqrt(hd))
  V  [2048,260]  = xT.T @ wv           (token-major; +bias, with a ones
                                        column per head -> vaug)
  per head h, key-block j (128 keys):
    S^T[k,q]     = kT_hj.T @ qT (q-window = 384 cols: blocks j..j+2)
    P^T          = exp(S^T) (bf16), zero band-complement triangles
                   (one strided DVE op)
    yT_psum[65,512] += vaug_hj.T @ P^T   (row 64 = softmax denominator)
  per (h, q-range g of 512), staggered across later ticks so no engine
  FIFO head-blocks:
    dn[1,512]  = denom row copied to partition 0 (ACT)
    rec        = reciprocal_approx_fast(dn) (custom DVE op)
    bc[64,512] = partition_broadcast(rec) (GpSimd)
    yT (bf16)  = yT_psum * bc (DVE)
  out[2048,1024] = yT.T @ wo  (partial; host adds across head-groups + bo)

Schedule: attention runs in g-major blocks (all heads for one 512-query
range) with crossing pV contributions deferred via retained P^T tiles
and replayed in the next block, so out-projection + output DMA fire at
every block tail, keeping PE duty (and the HAM clock gate) up. The
first block (EARLY) is interleaved into the qk/V projection of tokens
1024-2047 so ACT's exp work overlaps the PE-dense GEMM phase.
"""

import os
import sys

import numpy as np

if "/opt/trn_rl_repo" not in sys.path:
    sys.path.insert(0, "/opt/trn_rl_repo")

B, T, D = 2, 2048, 1024
H, W = 16, 256
HD = D // H          # 64
NCORES = 8
HPC = 4              # heads per core
FB = HPC * HD        # 256 f-columns per core

_STATE: dict = {}


def _build_module():
    import concourse.bacc as bacc
    import concourse.tile as tile
    from concourse import mybir

    dt = mybir.dt
    AF = mybir.ActivationFunctionType
    OP = mybir.AluOpType

    nc = bacc.Bacc(
        "TRN2",
        target_bir_lowering=False,
        debug=False,
        enable_asserts=False,
        num_devices=NCORES,
    )

    f32 = dt.float32
    f32r = dt.float32r
    bf16 = dt.bfloat16
    xT_d = nc.dram_tensor("xT", [D, T], bf16, kind="ExternalInput").ap()
    wqk_d = nc.dram_tensor("wqk", [D, 2 * FB], bf16, kind="ExternalInput").ap()
    bqk_d = nc.dram_tensor("bqk", [128, 4], f32, kind="ExternalInput").ap()
    wv_d = nc.dram_tensor("wv", [D, FB], bf16, kind="ExternalInput").ap()
    bvb_d = nc.dram_tensor("bvb", [128, HPC, HD], f32, kind="ExternalInput").ap()
    wo_d = nc.dram_tensor("wo", [FB, D], bf16, kind="ExternalInput").ap()
    tris_d = nc.dram_tensor("tris", [128, 256], bf16, kind="ExternalInput").ap()
    out_d = nc.dram_tensor("out_p", [T, D], bf16, kind="ExternalOutput").ap()

    KC = D // 128     # 8 contraction chunks
    NT = T // 128     # 16 token tiles / key blocks
    NQ = T // 512     # 4 q-ranges

    with tile.TileContext(nc) as tc:
        with (
            tc.tile_pool(name="const", bufs=1) as cpool,
            tc.tile_pool(name="work", bufs=3) as wpool,
            tc.tile_pool(name="ps", bufs=8, space="PSUM") as ppool,
        ):
            # ---- persistent SBUF ----
            # per-contraction-chunk tiles so each matmul depends only on
            # its own chunk's DMA, not the whole operand
            xTa_t = [cpool.tile([128, 1024], bf16, name=f"xTa{a}")
                     for a in range(KC)]
            xTb_t = [cpool.tile([128, 1024], bf16, name=f"xTb{a}")
                     for a in range(KC)]
            wqk_t = [cpool.tile([128, 2 * FB], bf16, name=f"wqk{a}")
                     for a in range(KC)]
            wv_t = [cpool.tile([128, FB], bf16, name=f"wv{a}")
                    for a in range(KC)]
            wo_sb = cpool.tile([128, 2, D], bf16)
            bqk_sb = cpool.tile([128, 4], f32)
            bvb_sb = cpool.tile([128, HPC, HD], f32)
            tris_sb = cpool.tile([128, 256], bf16)
            qkT_sb = cpool.tile([128, 4, T], bf16)
            vaug_sb = cpool.tile([128, NT, HPC, HD + 1], bf16)
            yTn_sb = cpool.tile([128, 2, T], bf16)

            # vaug ones columns via memset (DVE is idle during load)
            for h in range(HPC):
                nc.vector.memset(vaug_sb[:, :, h, HD:HD + 1], 1.0)

            # Two DMA queues (SP + ACT hwdge): group-0 operands (wqk, xTa)
            # lead both queues so the first matmul can start ~1us in;
            # constants follow, then second-half operands. ACT is idle
            # during the load phase.
            for a in range(KC):
                nc.sync.dma_start(wqk_t[a][:], wqk_d[a * 128:(a + 1) * 128, :])
                eng = nc.scalar if a % 2 == 0 else nc.gpsimd
                eng.dma_start(
                    xTa_t[a][:], xT_d[a * 128:(a + 1) * 128, 0:1024]
                )
            nc.sync.dma_start(bqk_sb[:], bqk_d[:])
            nc.sync.dma_start(bvb_sb[:], bvb_d[:])
            nc.sync.dma_start(tris_sb[:], tris_d[:])
            for a in range(KC):
                nc.sync.dma_start(wv_t[a][:], wv_d[a * 128:(a + 1) * 128, :])
            for a in range(KC):
                nc.gpsimd.dma_start(
                    xTb_t[a][:], xT_d[a * 128:(a + 1) * 128, 1024:2048]
                )
            for f in range(2):
                nc.sync.dma_start(wo_sb[:, f, :], wo_d[f * 128:(f + 1) * 128, :])

            # ---- qkT projection: [512, 2048] ----
            def qkT_pass(grp, xh_t, mm):
                # one m-pair pass of 4 PSUM tiles: enough bank distance to
                # avoid same-bank accumulation hazards, while leaving 4
                # ring slots for interleaved attention tiles. Pass mm=0
                # covers m={0,2} = q+k rows of heads 0-1, so their
                # attention can start after half the group.
                tiles = [(m, n) for m in (mm, mm + 2) for n in range(2)]
                ps_g = {
                    mn: ppool.tile([128, 512], f32, tag="ps",
                                   name=f"ps_qk{grp}_{mn[0]}_{mn[1]}")
                    for mn in tiles
                }
                for a in range(KC):
                    for (m, n) in tiles:
                        nc.tensor.matmul(
                            ps_g[(m, n)][:],
                            lhsT=wqk_t[a][:, m * 128:(m + 1) * 128],
                            rhs=xh_t[a][:, n * 512:(n + 1) * 512],
                            start=(a == 0),
                            stop=(a == KC - 1),
                        )
                for (m, n) in tiles:
                    nc.scalar.activation(
                        qkT_sb[:, m,
                               (2 * grp + n) * 512:(2 * grp + n + 1) * 512],
                        ps_g[(m, n)][:],
                        AF.Identity,
                        bias=bqk_sb[:, m:m + 1],
                    )

            def v_proj(t, xh_t):
                tl = t % 8
                ps_v = ppool.tile([128, HPC, HD], f32, tag="ps", name=f"ps_v_{t}")
                for a in range(KC):
                    nc.tensor.matmul(
                        ps_v[:],
                        lhsT=xh_t[a][:, tl * 128:(tl + 1) * 128],
                        rhs=wv_t[a][:],
                        start=(a == 0),
                        stop=(a == KC - 1),
                    )
                nc.vector.tensor_tensor(
                    out=vaug_sb[:, t, :, 0:HD],
                    in0=ps_v[:],
                    in1=bvb_sb[:],
                    op=OP.add,
                )

            qkT_pass(0, xTa_t, 0)
            for t in range(8):
                v_proj(t, xTa_t)

            # ---- attention: software-pipelined over flattened (h, j) ----
            # stage A:  paired S^T matmuls (j even + odd) into a 2-bank
            #           PSUM tile -> ONE exp over both halves (strided AP)
            #           -> band-complement triangle masks per half (DVE)
            # stage B:  pV matmuls
            # Step order interleaves an EARLY batch (all h, j<=3, pV
            # clipped to q-range g=0) before group-1 projection, so ACT's
            # exp work overlaps the PE-dense projection of tokens
            # 1024-2047. The clipped g=1 contributions of j=2,3 replay
            # from retained pT tiles at each head's first REST step.
            # The (h,g)-tail normalize chain (denom copy -> approx recip ->
            # partition broadcast -> multiply) is staggered across later
            # ticks via `post`, so each op's inputs are already complete
            # when it reaches its engine's strict-FIFO head — otherwise
            # the chain head-blocks the DVE/GpSimd queues that the
            # per-step mask ops need, stalling the PE ~13us per group.
            DELAY = 4
            EARLY_N = 4 * HPC
            # g-major blocks: all heads' steps for one 512-query range,
            # crossing pV contributions deferred to the next block (replay
            # from retained pT). out-projection + output DMA then fire at
            # every block tail, keeping PE duty (and the HAM clock) up
            # through the whole attention phase.
            steps = [(h, j, g) for g in range(NQ) for h in range(HPC)
                     for j in range(4 * g, min(4 * g + 4, NT))]
            pT_t = {}
            ps_y = {}
            post = {}

            def at_step(s, fn):
                post.setdefault(s, []).append(fn)

            o_tiles = {}

            def out_proj_chunk(mt, nn):
                # one (mt, nn) output tile: 2 accum matmuls + copy (+ DMA
                # when the mt row completes). Chunks are spread across
                # ticks so the PE burst doesn't starve the exp pipeline.
                if nn == 0:
                    o_tiles[mt] = wpool.tile(
                        [128, 2, 512], bf16, bufs=6,
                        name=f"o_{mt}", tag="o_sb",
                    )
                o_sb = o_tiles[mt]
                ps_o = ppool.tile(
                    [128, 512], f32, tag="ps", name=f"ps_o_{mt}_{nn}",
                )
                for fc in range(2):
                    nc.tensor.matmul(
                        ps_o[:],
                        lhsT=yTn_sb[:, fc, mt * 128:(mt + 1) * 128],
                        rhs=wo_sb[:, fc, nn * 512:(nn + 1) * 512],
                        start=(fc == 0),
                        stop=(fc == 1),
                    )
                if (mt + nn) % 2 == 0:
                    nc.vector.tensor_copy(out=o_sb[:, nn, :], in_=ps_o[:])
                else:
                    nc.scalar.copy(o_sb[:, nn, :], ps_o[:])
                if nn == 1:
                    del o_tiles[mt]
                    eng = nc.sync if mt % 2 == 0 else nc.gpsimd
                    eng.dma_start(
                        out_d[mt * 128:(mt + 1) * 128, :], o_sb[:]
                    )

            def stage_a(k):
                h, j, _g = steps[k]
                po = 64 * (h % 2)
                qwin = min(384, T - 128 * j)
                ps_s = ppool.tile([128, 384], f32, tag="ps",
                                  name=f"ps_s_{k}")
                nc.tensor.matmul(
                    ps_s[:, :qwin],
                    lhsT=qkT_sb[po:po + 64, 2 + h // 2,
                                j * 128:(j + 1) * 128],
                    rhs=qkT_sb[po:po + 64, h // 2,
                               j * 128:j * 128 + qwin],
                    start=True,
                    stop=True,
                )
                pT = wpool.tile([128, 384], bf16, bufs=16,
                                name=f"pT_{k}", tag="pT")
                nc.scalar.activation(pT[:, :qwin], ps_s[:, :qwin], AF.Exp)
                if qwin == 384:
                    pv = pT[:].rearrange("p (a b) -> p a b", a=3)[:, 0:3:2, :]
                    tv = tris_sb[:].rearrange("p (a b) -> p a b", a=2)
                    nc.vector.tensor_tensor(
                        out=pv, in0=pv, in1=tv, op=OP.mult,
                    )
                else:
                    nc.vector.tensor_tensor(
                        out=pT[:, 0:128], in0=pT[:, 0:128],
                        in1=tris_sb[:, 0:128], op=OP.mult,
                    )
                pT_t[(h, j)] = pT

            def stage_b(idx, pidx):
                h, j, g = steps[idx]
                po = 64 * (h % 2)
                qwin = min(384, T - 128 * j)
                if g >= 1 and j == 4 * g:
                    # replay the parts of the previous block's crossing
                    # key-blocks that were clipped to their own g
                    ps_y[(h, g)] = ppool.tile(
                        [65, 512], f32, tag="ps", name=f"ps_y_{h}_{g}"
                    )
                    for jr in (4 * g - 2, 4 * g - 1):
                        pTr = pT_t.pop((h, jr))
                        c0 = 512 * g - 128 * jr
                        c1 = min(384, 512 * (g + 1) - 128 * jr)
                        nc.tensor.matmul(
                            ps_y[(h, g)][:, 0:c1 - c0],
                            lhsT=vaug_sb[:, jr, h, :],
                            rhs=pTr[:, c0:c1],
                            start=(jr == 4 * g - 2),
                            stop=False,
                            skip_group_check=True,
                        )
                pT = pT_t[(h, j)]
                if not (j >= 4 * g + 2 and g < NQ - 1):
                    del pT_t[(h, j)]
                if True:
                    c0 = 0
                    c1 = min(qwin, 512 * (g + 1) - 128 * j)
                    if (h, g) not in ps_y:
                        ps_y[(h, g)] = ppool.tile(
                            [65, 512], f32, tag="ps", name=f"ps_y_{h}_{g}"
                        )
                    first = (g == 0 and j == 0)
                    last = (j == min(NT - 1, 4 * g + 3))
                    d0 = 128 * j - 512 * g
                    nc.tensor.matmul(
                        ps_y[(h, g)][:, d0:d0 + (c1 - c0)],
                        lhsT=vaug_sb[:, j, h, :],
                        rhs=pT[:, c0:c1],
                        start=first,
                        stop=last,
                        skip_group_check=True,
                    )
                    if not last:
                        return
                    yps = ps_y.pop((h, g))
                    # reciprocal_approx_fast and partition_broadcast both
                    # require partition base 0: stage the denominator row
                    # at partition 0 first.
                    dn = wpool.tile([1, 512], f32, bufs=4,
                                    name=f"dn_{h}_{g}", tag="dn")
                    rec = wpool.tile([1, 512], f32, bufs=4,
                                     name=f"rec_{h}_{g}", tag="rec")
                    bc_sb = wpool.tile([64, 512], f32, bufs=3,
                                       name=f"bc_{h}_{g}", tag="bc")

                    def dn_copy(dn=dn, yps=yps):
                        nc.scalar.copy(dn[:], yps[64:65, :])

                    def do_recip(rec=rec, dn=dn):
                        nc.vector.reciprocal_approx_fast(rec[:], dn[:])

                    def do_bcast(bc_sb=bc_sb, rec=rec):
                        nc.gpsimd.partition_broadcast(bc_sb[:], rec[0:1, :])

                    def do_mult(yps=yps, bc_sb=bc_sb, po=po, h=h, g=g):
                        nc.vector.tensor_tensor(
                            out=yTn_sb[po:po + 64, h // 2,
                                       g * 512:(g + 1) * 512],
                            in0=yps[0:64, :],
                            in1=bc_sb[:],
                            op=OP.mult,
                        )

                    at_step(pidx + 1, dn_copy)
                    at_step(pidx + 2, do_recip)
                    at_step(pidx + 3, do_bcast)
                    at_step(pidx + 4, do_mult)
                    if h == HPC - 1:
                        for i, (mt, nn) in enumerate(
                                (mt, nn)
                                for mt in range(4 * g, 4 * g + 4)
                                for nn in range(2)):
                            at_step(
                                pidx + 4 + i // 2,
                                lambda mt=mt, nn=nn: out_proj_chunk(mt, nn),
                            )

            units = ["s"] * 8
            units += [lambda: qkT_pass(0, xTa_t, 1)]
            units += ["s"] * 8
            units += [lambda: qkT_pass(1, xTb_t, 0),
                      lambda: qkT_pass(1, xTb_t, 1)]
            units += [(lambda t=t: v_proj(t, xTb_t)) for t in range(8, NT)]
            units += ["s"] * (len(steps) - EARLY_N)
            total_ticks = len(units) + DELAY + 10
            a_i = 0
            b_i = 0
            for tick in range(total_ticks):
                u = units[tick] if tick < len(units) else None
                is_emit = u is not None and u != "s"
                if u == "s":
                    stage_a(a_i)
                    a_i += 1
                elif is_emit:
                    u()
                if b_i < a_i and (a_i - b_i > DELAY or is_emit
                                  or u is None or a_i == len(steps)):
                    stage_b(b_i, tick)
                    b_i += 1
                for fn in post.pop(tick, []):
                    fn()

    nc.compile()
    from concourse.bass_interp import get_hw_module

    nc.m = get_hw_module(nc.m)
    return nc


def _shard_inputs(x, Wqkv, bqkv, Wo, bo):
    import ml_dtypes

    bfdt = ml_dtypes.bfloat16

    x = np.asarray(x, np.float32)
    Wqkv = np.asarray(Wqkv, np.float32)
    bqkv = np.asarray(bqkv, np.float32)
    Wo = np.asarray(Wo, np.float32)

    scale = 1.0 / np.sqrt(np.float32(HD))
    c_idx = np.arange(128)[:, None]
    u_idx = np.arange(128)[None, :]
    tri0 = (u_idx >= c_idx).astype(np.float32)   # keys block j vs q block j
    tri1 = (u_idx < c_idx).astype(np.float32)    # keys block j vs q block j+2
    tris = np.concatenate([tri0, tri1], axis=1)

    in_maps = []
    for c in range(NCORES):
        b, hg = divmod(c, HPC)
        r0 = hg * FB
        Wq = Wqkv[r0:r0 + FB] * scale
        Wk = Wqkv[D + r0:D + r0 + FB]
        Wv = Wqkv[2 * D + r0:2 * D + r0 + FB]
        bq = bqkv[r0:r0 + FB] * scale
        bk = bqkv[D + r0:D + r0 + FB]
        bv = bqkv[2 * D + r0:2 * D + r0 + FB]
        in_maps.append({
            "xT": np.ascontiguousarray(x[b].T).astype(bfdt),
            "wqk": np.ascontiguousarray(
                np.concatenate([Wq, Wk], 0).T).astype(bfdt),
            "bqk": np.ascontiguousarray(
                np.concatenate([bq, bk]).reshape(4, 128).T),
            "wv": np.ascontiguousarray(Wv.T).astype(bfdt),
            "bvb": np.ascontiguousarray(
                np.broadcast_to(bv[None, :], (128, FB))
            ).reshape(128, HPC, HD),
            "wo": np.ascontiguousarray(Wo[:, r0:r0 + FB].T).astype(bfdt),
            "tris": tris.astype(bfdt),
            "vone": np.ones((128, 64), bfdt),
        })
    return in_maps


def kernel(x, Wqkv, bqkv, Wo, bo):
    from concourse import bass_utils

    if "nc" not in _STATE:
        _STATE["nc"] = _build_module()
    nc = _STATE["nc"]

    in_maps = _shard_inputs(x, Wqkv, bqkv, Wo, bo)
    trace = bool(os.environ.get("TRNKERN_TRACE"))
    res = bass_utils.run_bass_kernel_spmd(
        nc,
        in_maps,
        core_ids=list(range(NCORES)),
        trace=trace,
    )
    _STATE["last"] = res

    bo = np.asarray(bo, np.float32)
    out = np.empty((B, T, D), np.float32)
    for b in range(B):
        acc = res.results[b * HPC]["out_p"].astype(np.float32)
        for hg in range(1, HPC):
            acc = acc + res.results[b * HPC + hg]["out_p"].astype(np.float32)
        out[b] = acc + bo[None, :]
    return out
